# revision 9
# baseline (speedup 1.0000x reference)
"""Group MoE layer (2 groups x 4 experts, top-1 group / top-2 expert routing)
on 8 Trainium2 NeuronCores via expert parallelism.

Strategy:
  - Host computes the (tiny) routing: language-gate argmax over groups,
    per-group expert top-k + softmax weights.
  - Tokens are dispatched by (group, expert) assignment: core c = g*4+e
    receives exactly the tokens routed to expert (g, e), padded to a common
    capacity C (SPMD: all cores run the same program).
  - Each core runs the dense FFN for its expert:
        Y^T = W2 @ relu(W1 @ X^T + b1) + b2      (tokens in the moving dim)
    with bf16 weights/activations and fp32 PSUM accumulation.
  - Capacity is padded to 32 (not 128) and split into token blocks that are
    all >=128 wide: a matmul with <128 moving columns is LDWEIGHTS-bound
    (~56ns floor vs n*0.417ns), so [512, 416, 128] beats [512, 512, 32].
  - All bulk DMA goes need-ordered on one queue (x blk0, W1 chunks small
    first, then the rest, W2 last); biases ride the scalar queue. This keeps
    the critical first 1.25MB (x0 + first W1 chunk) uncontended so the PE
    starts ~6us in, instead of ~17us when W2's 8.4MB shares the pipe.
  - A short burst of dummy matmuls at t=0 warms the PE HAM clock gate
    (cold PE runs at 1.2GHz for the first ~3.4us window).
  - Host scatter-adds the weighted expert outputs back into the full output.
"""

import numpy as np
import ml_dtypes

import concourse.bacc as bacc
import concourse.mybir as mybir
from concourse import tile
from concourse import bass_utils

B, L, D, H = 2, 2048, 1024, 4096
G, E = 2, 4
NCORES = G * E
PART = 128

# W1 chunk sizes in h-tiles (sum = H/128 = 32): small first so the first
# relu's weights land fast, big later to keep dma_start trigger count low.
W1_CHUNKS = [1, 1, 1, 1, 2, 2, 4, 4, 8, 8]
W2_CHUNKS = [4, 4, 4, 4, 4, 4, 4, 4]

_BF16 = ml_dtypes.bfloat16

_program_cache: dict[tuple, object] = {}


def _blocks(C: int) -> list[int]:
    """Split capacity C (multiple of 4) into moving-dim blocks, all >=128
    (below 128 columns a matmul hits the LDWEIGHTS floor) and <=512 (PSUM
    bank limit). A 384-token first block shrinks the critical startup DMA
    (x block 0) while still pacing PE weight consumption below the DMA
    stream rate; the small last block shrinks the drain tail."""
    assert C >= 128 and C % 4 == 0
    out = []
    rem = C
    if rem >= 384 + 512:
        out.append(384)
        rem -= 384
    while rem > 640:
        out.append(512)
        rem -= 512
    if rem > 512:
        out.extend([rem - 128, 128])
    else:
        out.append(rem)
    return out


def _build(C: int, d: int = D, h: int = H):
    """Build + compile the per-core expert FFN program for capacity C."""
    key = (C, d, h)
    if key in _program_cache:
        return _program_cache[key]

    nd = d // PART
    nh = h // PART
    ns = _blocks(C)
    nblk = len(ns)
    xoff = np.concatenate([[0], np.cumsum(ns)])  # token offsets per block

    # chunk maps: h-tile index -> (chunk idx, h-tile offset inside chunk)
    w1_start = np.concatenate([[0], np.cumsum(W1_CHUNKS)])
    w2_start = np.concatenate([[0], np.cumsum(W2_CHUNKS)])
    assert w1_start[-1] == nh and w2_start[-1] == nh

    bf16 = mybir.dt.bfloat16
    f32 = mybir.dt.float32

    nc = bacc.Bacc("TRN2", target_bir_lowering=False, debug=False,
                   num_devices=NCORES)

    # Merged-tile layouts: per partition row everything is contiguous, so
    # each dma_start is 128 large descriptors.
    xt = nc.dram_tensor("xt", [PART, nd * C], bf16, kind="ExternalInput")
    w1ts = [nc.dram_tensor(f"w1t{i}", [PART, nd * ch * PART], bf16,
                           kind="ExternalInput")
            for i, ch in enumerate(W1_CHUNKS)]
    w2ts = [nc.dram_tensor(f"w2t{i}", [PART, ch * d], bf16,
                           kind="ExternalInput")
            for i, ch in enumerate(W2_CHUNKS)]
    b1t = nc.dram_tensor("b1t", [PART, nh], f32, kind="ExternalInput")
    b2t = nc.dram_tensor("b2t", [PART, nd], f32, kind="ExternalInput")
    yt = nc.dram_tensor("yt", [PART, nd * C], f32, kind="ExternalOutput")

    with tile.TileContext(nc) as tc:
        with (
            tc.tile_pool(name="wpool", bufs=1) as wpool,
            tc.tile_pool(name="h1pool", bufs=nh) as h1pool,
            tc.tile_pool(name="ypool", bufs=1) as ypool,
            tc.tile_pool(name="ps1", bufs=4, space="PSUM") as ps1,
            tc.tile_pool(name="ps2", bufs=4, space="PSUM") as ps2,
        ):
            # --- PE warm-up: the HAM clock gate keeps a cold PE at 1.2GHz
            # until ~3.4us of sustained activity. Burn dummy matmuls on a
            # zeroed tile while the first DMAs are in flight so the real
            # matmuls start at 2.4GHz.
            warm = wpool.tile([PART, 256], bf16, tag="warm")
            nc.vector.memset(warm[:, :], 0.0)
            wps = ps2.tile([PART, 512], f32, tag="ps2")
            NWARM = 21
            for i in range(NWARM):
                nc.tensor.matmul(wps[:, :256], warm[:, :PART], warm[:, :],
                                 start=(i == 0), stop=(i == NWARM - 1))

            # Biases on the scalar engine's HWDGE queue (tiny, parallel to
            # the bulk stream; needed by the first relu).
            b1_sb = wpool.tile([PART, nh], f32, tag="b1")
            nc.scalar.dma_start(out=b1_sb[:, :], in_=b1t.ap()[:, :])
            b2_sb = wpool.tile([PART, nd], f32, tag="b2")
            nc.scalar.dma_start(out=b2_sb[:, :], in_=b2t.ap()[:, :])

            # Bulk loads, strictly need-ordered on the sync queue: the PE
            # can start once x block 0 + W1 chunk 0 land; W2 (first needed
            # ~60us in) goes last so it never contends with the critical
            # prefix.
            x_sb = []
            for blk, n in enumerate(ns):
                t = wpool.tile([PART, nd * n], bf16, tag=f"x_{blk}")
                x_sb.append(t)
            w1_sb = []
            for i, ch in enumerate(W1_CHUNKS):
                t = wpool.tile([PART, nd * ch * PART], bf16, tag=f"w1_{i}")
                w1_sb.append(t)
            w2_sb = []
            for i, ch in enumerate(W2_CHUNKS):
                t = wpool.tile([PART, ch * d], bf16, tag=f"w2_{i}")
                w2_sb.append(t)

            nc.sync.dma_start(out=x_sb[0][:, :], in_=xt.ap()[:, :nd * ns[0]])
            for i in range(len(W1_CHUNKS)):
                nc.sync.dma_start(out=w1_sb[i][:, :], in_=w1ts[i].ap()[:, :])
            # W2 before the later x blocks: mm2 of block 0 touches ALL of W2
            # within its first d-tile (~60us in); x block 1 isn't needed
            # until mm1 of block 1 (~97us in).
            for i in range(len(W2_CHUNKS)):
                nc.sync.dma_start(out=w2_sb[i][:, :], in_=w2ts[i].ap()[:, :])
            for blk in range(1, nblk):
                nc.sync.dma_start(
                    out=x_sb[blk][:, :],
                    in_=xt.ap()[:, nd * xoff[blk]:nd * xoff[blk + 1]])

            for blk in range(nblk):
                n = ns[blk]
                h1_tiles = []
                for hi in range(nh):
                    hc = int(np.searchsorted(w1_start, hi, "right")) - 1
                    ho = hi - w1_start[hc]
                    chw = W1_CHUNKS[hc] * PART
                    ps = ps1.tile([PART, 512], f32, tag="ps1")
                    for di in range(nd):
                        nc.tensor.matmul(
                            ps[:, :n],
                            w1_sb[hc][:, di * chw + ho * PART:
                                      di * chw + (ho + 1) * PART],
                            x_sb[blk][:, di * n:(di + 1) * n],
                            start=(di == 0), stop=(di == nd - 1),
                        )
                    h1 = h1pool.tile([PART, 512], bf16, tag="h1")
                    nc.scalar.activation(h1[:, :n], ps[:, :n],
                                         mybir.ActivationFunctionType.Relu,
                                         bias=b1_sb[:, hi:hi + 1], scale=1.0)
                    h1_tiles.append(h1)
                y = ypool.tile([PART, nd * 512], f32, tag="y")
                for di in range(nd):
                    ps = ps2.tile([PART, 512], f32, tag="ps2")
                    for hi in range(nh):
                        gi = int(np.searchsorted(w2_start, hi, "right")) - 1
                        hj = hi - w2_start[gi]
                        nc.tensor.matmul(
                            ps[:, :n],
                            w2_sb[gi][:, hj * d + di * PART:
                                      hj * d + (di + 1) * PART],
                            h1_tiles[hi][:, :n],
                            start=(hi == 0), stop=(hi == nh - 1),
                        )
                    nc.vector.tensor_scalar_add(
                        y[:, di * n:(di + 1) * n], ps[:, :n],
                        b2_sb[:, di:di + 1])
                    # drain each d-tile as soon as it's ready (overlaps mm2,
                    # shrinks the end-of-kernel tail to one small transfer)
                    nc.sync.dma_start(
                        out=yt.ap()[:, nd * xoff[blk] + di * n:
                                    nd * xoff[blk] + (di + 1) * n],
                        in_=y[:, di * n:(di + 1) * n])

    nc.compile()
    _program_cache[key] = nc
    return nc


def _route(x, bn, Wlg, blg, Wg, k):
    """Numpy replica of the reference routing. Returns per-(g,e) assignment."""
    glog = bn @ Wlg.T + blg                       # (N, G)
    sel_group = np.argmax(glog, axis=1)           # (N,)
    assign = []
    for g in range(Wg.shape[0]):
        logits = x @ Wg[g].T                      # (N, E)
        order = np.argsort(-logits, axis=1, kind="stable")
        sel = order[:, :k]                        # (N, k)
        top = np.take_along_axis(logits, sel, axis=1).astype(np.float32)
        m = top.max(axis=1, keepdims=True)
        ex = np.exp(top - m)
        w = ex / ex.sum(axis=1, keepdims=True)    # (N, k)
        assign.append((sel, w))
    return sel_group, assign


def _pack_x(X, d, ns):
    """(C, d) fp32 -> [128, nd*C] bf16 merged-tile layout, block-major:
    per partition row: [blk][di][token]."""
    nd = d // PART
    xt = X.T.astype(_BF16)                        # (d, C)
    parts = []
    c0 = 0
    for n in ns:
        blk = xt[:, c0:c0 + n].reshape(nd, PART, n).transpose(1, 0, 2)
        parts.append(blk.reshape(PART, nd * n))
        c0 += n
    return np.ascontiguousarray(np.concatenate(parts, axis=1))


def _pack_w1(W1e, d):
    """(h, d) -> per-chunk [128, nd*ch*128] bf16: per partition row
    [di][h cols of chunk]."""
    nd = d // PART
    w = W1e.T.astype(_BF16)                       # (d, h)
    outs = []
    h0 = 0
    for ch in W1_CHUNKS:
        cw = ch * PART
        c = w[:, h0:h0 + cw].reshape(nd, PART, cw).transpose(1, 0, 2)
        outs.append(np.ascontiguousarray(c.reshape(PART, nd * cw)))
        h0 += cw
    return outs


def _pack_w2(W2e, d):
    """(d, h) -> per-chunk [128, ch*d] bf16: per partition row
    [hj][d cols]."""
    w = W2e.T.astype(_BF16)                       # (h, d)
    outs = []
    h0 = 0
    for ch in W2_CHUNKS:
        c = w[h0 * PART:(h0 + ch) * PART, :].reshape(ch, PART, d)
        outs.append(np.ascontiguousarray(
            c.transpose(1, 0, 2).reshape(PART, ch * d)))
        h0 += ch
    return outs


def _unpack_y(yt, d, ns):
    """[128, nd*C] f32 -> (d, C)."""
    nd = d // PART
    out = np.empty((d, int(sum(ns))), np.float32)
    c0 = 0
    o0 = 0
    for n in ns:
        blk = yt[:, o0:o0 + nd * n].reshape(PART, nd, n).transpose(1, 0, 2)
        out[:, c0:c0 + n] = blk.reshape(d, n)
        c0 += n
        o0 += nd * n
    return out


def kernel(**inputs) -> np.ndarray:
    xs = np.asarray(inputs["xs"], np.float32)
    bn = np.asarray(inputs["bottle_neck"], np.float32)
    Wlg = np.asarray(inputs["Wlg"], np.float32)
    blg = np.asarray(inputs["blg"], np.float32)
    Wg = np.asarray(inputs["Wg"], np.float32)
    W1 = np.asarray(inputs["W1"], np.float32)
    b1 = np.asarray(inputs["b1"], np.float32)
    W2 = np.asarray(inputs["W2"], np.float32)
    b2 = np.asarray(inputs["b2"], np.float32)
    k = int(np.asarray(inputs["top_k"]))

    Bx, Lx, d = xs.shape
    hdim = W1.shape[2]
    N = Bx * Lx
    nh = hdim // PART
    nd = d // PART
    x = xs.reshape(N, d)
    bnf = bn.reshape(N, d)

    sel_group, assign = _route(x, bnf, Wlg, blg, Wg, k)

    # Token sets per (group, expert) core.
    idxs, wgts = [], []
    for c in range(NCORES):
        g, e = divmod(c, E)
        sel, w = assign[g]
        mask = (sel_group == g)[:, None] & (sel == e)
        rows, cols = np.nonzero(mask)
        idxs.append(rows)
        wgts.append(w[rows, cols])

    cnt_max = max(len(i) for i in idxs)
    C = max(PART, -(-cnt_max // 4) * 4)           # pad capacity to 4
    ns = _blocks(C)

    nc = _build(C, d, hdim)

    in_maps = []
    for c in range(NCORES):
        g, e = divmod(c, E)
        cnt = len(idxs[c])
        X = np.zeros((C, d), np.float32)
        if cnt:
            X[:cnt] = x[idxs[c]]
        m = {
            "xt": _pack_x(X, d, ns),
            "b1t": np.ascontiguousarray(b1[g, e].reshape(nh, PART).T),
            "b2t": np.ascontiguousarray(b2[g, e].reshape(nd, PART).T),
        }
        for i, arr in enumerate(_pack_w1(W1[g, e], d)):
            m[f"w1t{i}"] = arr
        for i, arr in enumerate(_pack_w2(W2[g, e], d)):
            m[f"w2t{i}"] = arr
        in_maps.append(m)

    res = bass_utils.run_bass_kernel_spmd(nc, in_maps, core_ids=list(range(NCORES)))

    out = np.zeros((N, d), np.float32)
    for c in range(NCORES):
        cnt = len(idxs[c])
        if cnt == 0:
            continue
        yc = _unpack_y(res.results[c]["yt"], d, ns)[:, :cnt].T
        out[idxs[c]] += wgts[c][:, None] * yc
    return out.reshape(Bx, Lx, d).astype(np.float32)
